# revision 1
# baseline (speedup 1.0000x reference)
"""Trainium2 Bass kernel for nn_Concat_73607149519362.

Math (decomposed concat-MLP attention score):
    score[b, d, e] = dec[b, d] @ w_dec + enc[b, e] @ w_enc + bias

Sharding: data-parallel over batch, 32 batches / 8 cores = 4 per core.
Raw bass with explicit semaphores (the Tile framework's attached
sync_info waits don't encode on this toolchain).

Per-core per-batch pipeline:
  SP  : enc (2x2MB) + dec (2MB) input DMAs, double-buffered slots.
        DRAM views are p-major so every partition reads one contiguous
        16-32KB run (cheap descriptors).
  DVE : one fp32 tensor_tensor multiply per 128-row chunk against the
        broadcast weight row; also the free-axis reduce for the last 3
        dec chunks (engine balance).
  ACT : activation(Copy, accum_out) reduces for the 8 enc chunks + 1 dec
        chunk; the enc_proj row copy (PSUM->SBUF, permuted view, bias
        folded in); the 4 output builds (Identity, per-partition bias =
        dec projection); the output DMA issue.
  PE  : 8 tiny transposes flatten enc_proj columns into a (1, enc) PSUM
        row; 2 ones outer-product matmuls rebroadcast it to (128, enc)
        PSUM for the ACT builds.
  Pool: one-time broadcast loads of weights/bias/identity/ones.
"""

import os
from contextlib import ExitStack

os.environ.setdefault("JAX_PLATFORMS", "axon")

import numpy as np

import concourse.bass as bass
import concourse.mybir as mybir
from concourse.bass_utils import run_bass_kernel_spmd

B, DEC, ENC, DIM = 32, 512, 1024, 1024
NCORES = 8
BPC = B // NCORES  # batches per core

F32 = mybir.dt.float32
P = 128
NSCR = 4  # rotating DVE-product scratch slots


def _build(bpc=BPC, dec=DEC, enc=ENC, dim=DIM):
    nc = bass.Bass("TRN2")
    dec_h = nc.dram_tensor("dec_in", [bpc * dec, dim], F32, kind="ExternalInput")
    enc_h = nc.dram_tensor("enc_in", [bpc * enc, dim], F32, kind="ExternalInput")
    wenc_h = nc.dram_tensor("w_enc", [1, dim], F32, kind="ExternalInput")
    wdec_h = nc.dram_tensor("w_dec", [1, dim], F32, kind="ExternalInput")
    bias_h = nc.dram_tensor("bias", [1, 1], F32, kind="ExternalInput")
    ident_h = nc.dram_tensor("ident_in", [P, P], F32, kind="ExternalInput")
    ones_h = nc.dram_tensor("ones_in", [1, P], F32, kind="ExternalInput")
    out_h = nc.dram_tensor("out", [bpc * dec, enc], F32, kind="ExternalOutput")

    te = enc // P  # enc 128-row chunks per batch
    td = dec // P  # dec 128-row chunks per batch
    assert te % 2 == 0
    nprod = te + td
    ndve_red = min(3, td)  # dec reduces done on DVE (engine balance)
    nact_red = nprod - ndve_red  # reduces done on ACT
    DV = nprod + ndve_red  # DVE s_mult increments per batch
    A = nact_red + 1 + td  # ACT s_acc increments per batch
    nblk = min(512, enc)
    nh = enc // nblk
    eh = te // 2  # enc chunks per half-load

    # p-major views: within a batch, partition p owns rows p*t..p*t+t-1,
    # i.e. one contiguous run per partition per DMA.
    dec_r = dec_h.ap().rearrange("(b p t) d -> b p t d", p=P, t=td)
    enc_r = enc_h.ap().rearrange("(b p t) d -> b p t d", p=P, t=te)
    out_r = out_h.ap().rearrange("(b p t) e -> b p t e", p=P, t=td)

    # DVE op index (1-based s_mult count) for the multiply of chunk k:
    # order: m0..m(nact_red-1), then (mult, reduce) pairs for DVE chunks.
    def mult_count(b, k):
        if k < nact_red:
            return DV * b + k + 1
        return DV * b + nact_red + 2 * (k - nact_red) + 1

    def act_accum_count(b, k):  # ACT s_acc count after accum of chunk k
        return A * b + k + 1

    with ExitStack() as ctx:

        def sb(name, shape):
            return ctx.enter_context(nc.sbuf_tensor(name, shape, F32))

        w_enc_b = sb("w_enc_b", [P, dim])
        w_dec_b = sb("w_dec_b", [P, dim])
        bias_b = sb("bias_b", [P, 1])
        ident = sb("ident", [P, P])
        ones_row = sb("ones_row", [1, P])
        enc_t = [sb(f"enc_t{i}", [P, te, dim]) for i in range(2)]
        dec_t = [sb(f"dec_t{i}", [P, td, dim]) for i in range(2)]
        scr = [sb(f"scr{i}", [P, dim]) for i in range(NSCR)]
        eproj = [sb(f"eproj{i}", [P, te]) for i in range(2)]
        dproj = [sb(f"dproj{i}", [P, td]) for i in range(2)]
        enc_row = [sb(f"enc_row{i}", [1, enc]) for i in range(2)]
        out_t = [sb(f"out_t{i}", [P, td, enc]) for i in range(2)]
        tp_row = ctx.enter_context(nc.psum_tensor("tp_row", [1, enc], F32))
        ebc = ctx.enter_context(nc.psum_tensor("ebc", [P, enc], F32))

        s_gp = ctx.enter_context(nc.semaphore(name="s_gp"))
        # enc load groups: two 1-chunk groups first (fast compute start),
        # then 2-chunk groups
        egrp = [(0, 1), (1, 2)] + [(lo, lo + 2) for lo in range(2, te, 2)]
        nqe = len(egrp)
        nqd = (td + 1) // 2  # dec load groups
        s_enc = [
            [ctx.enter_context(nc.semaphore(name=f"s_enc{i}{h}")) for h in range(nqe)]
            for i in range(2)
        ]
        s_dec = [
            [ctx.enter_context(nc.semaphore(name=f"s_dec{i}{h}")) for h in range(nqd)]
            for i in range(2)
        ]
        s_mult = ctx.enter_context(nc.semaphore(name="s_mult"))
        s_acc = ctx.enter_context(nc.semaphore(name="s_acc"))
        s_pe = ctx.enter_context(nc.semaphore(name="s_pe"))
        s_outdma = [
            ctx.enter_context(nc.semaphore(name=f"s_outdma{i}")) for i in range(2)
        ]

        with nc.Block(no_gpsimd_drain=True) as block:

            @block.sync
            def _(sync):
                # issued DMA completion points, for depth-2 issue pipelining
                issued = []

                def issue(dma_fn, sem, val, war):
                    if war is not None:
                        sync.wait_ge(s_mult, war)
                    if len(issued) >= 3:
                        psem, pval = issued[-3]
                        sync.wait_ge(psem, pval)
                    dma_fn().then_inc(sem, 16)
                    issued.append((sem, val))

                for b in range(bpc):
                    use = 16 * (b // 2 + 1)
                    for q in range(nqe):
                        lo, hi = egrp[q]
                        war = (
                            mult_count(b - 2, hi - 1) if b >= 2 else None
                        )
                        issue(
                            lambda lo=lo, hi=hi, b=b: sync.dma_start(
                                enc_t[b % 2].ap()[:, lo:hi, :],
                                enc_r[b][:, lo:hi, :],
                            ),
                            s_enc[b % 2][q],
                            use,
                            war,
                        )
                    for q in range(nqd):
                        lo, hi = 2 * q, min(2 * q + 2, td)
                        war = (
                            mult_count(b - 2, te + hi - 1) if b >= 2 else None
                        )
                        issue(
                            lambda lo=lo, hi=hi, b=b: sync.dma_start(
                                dec_t[b % 2].ap()[:, lo:hi, :],
                                dec_r[b][:, lo:hi, :],
                            ),
                            s_dec[b % 2][q],
                            use,
                            war,
                        )


            @block.gpsimd
            def _(gpsimd):
                gpsimd.dma_start(
                    w_enc_b.ap(), wenc_h.ap().to_broadcast((P, dim))
                ).then_inc(s_gp, 16)
                gpsimd.dma_start(
                    w_dec_b.ap(), wdec_h.ap().to_broadcast((P, dim))
                ).then_inc(s_gp, 16)
                gpsimd.wait_ge(s_gp, 32)  # settle: make 32 a valid wait point
                gpsimd.dma_start(
                    bias_b.ap(), bias_h.ap().to_broadcast((P, 1))
                ).then_inc(s_gp, 16)
                gpsimd.wait_ge(s_gp, 48)  # settle: make 48 a valid wait point
                gpsimd.dma_start(ident.ap(), ident_h.ap()).then_inc(s_gp, 16)
                gpsimd.dma_start(ones_row.ap(), ones_h.ap()).then_inc(s_gp, 16)
                # ship outputs as their builds finish (ACT stays compute-only)
                for b in range(bpc):
                    if b < bpc - 1:
                        gpsimd.wait_ge(s_acc, A * (b + 1))
                        nc.gpsimd.dma_start(
                            out_r[b], out_t[b % 2].ap()
                        ).then_inc(s_outdma[b % 2], 16)
                    else:
                        # tail: slice the last batch so it drains early
                        for t in range(td):
                            gpsimd.wait_ge(
                                s_acc, A * b + nact_red + 1 + t + 1
                            )
                            nc.gpsimd.dma_start(
                                out_r[b][:, t, :], out_t[b % 2].ap()[:, t, :]
                            ).then_inc(s_outdma[b % 2], 16)

            @block.vector
            def _(vector):
                for b in range(bpc):
                    for k in range(nprod):
                        if b == 0 and k == 0:
                            vector.wait_ge(s_gp, 32)  # weight rows loaded
                        if k < te:
                            for qi, (lo, hi) in enumerate(egrp):
                                if k == lo:
                                    vector.wait_ge(
                                        s_enc[b % 2][qi], 16 * (b // 2 + 1)
                                    )
                        if k >= te and (k - te) % 2 == 0:
                            vector.wait_ge(
                                s_dec[b % 2][(k - te) // 2], 16 * (b // 2 + 1)
                            )
                        g = nprod * b + k  # global mult index -> scratch slot
                        if g >= NSCR and g % 2 == 0:
                            # cover the slots of this mult and the next one
                            need = 0
                            for gg in (g - NSCR, g + 1 - NSCR):
                                if gg >= 0:
                                    b2, k2 = divmod(gg, nprod)
                                    if k2 < nact_red:
                                        need = max(need, act_accum_count(b2, k2))
                            if need:
                                vector.wait_ge(s_acc, need)
                        if k < te:
                            src, wsrc = enc_t[b % 2].ap()[:, k, :], w_enc_b
                        else:
                            src, wsrc = dec_t[b % 2].ap()[:, k - te, :], w_dec_b
                        nc.vector.tensor_tensor(
                            out=scr[g % NSCR].ap(),
                            in0=src,
                            in1=wsrc.ap(),
                            op=mybir.AluOpType.mult,
                        ).then_inc(s_mult, 1)
                        if k >= nact_red:
                            # reduce this dec chunk ourselves (engine balance)
                            if b >= 2 and k == nact_red:
                                # WAR: dproj slot free once b-2's builds read it
                                vector.wait_ge(s_acc, A * (b - 1))
                            # self-wait: our multiply's writes must retire
                            vector.wait_ge(s_mult, mult_count(b, k))
                            nc.vector.tensor_reduce(
                                out=dproj[b % 2].ap()[:, k - te : k - te + 1],
                                in_=scr[g % NSCR].ap(),
                                axis=mybir.AxisListType.X,
                                op=mybir.AluOpType.add,
                            ).then_inc(s_mult, 1)

            @block.scalar
            def _(scalar):
                for b in range(bpc):
                    if b >= 2:
                        # WAR: eproj/dproj slot free once batch b-2's PE used it.
                        scalar.wait_ge(s_pe, 2 * (b - 1))
                    for k in range(nact_red):
                        if k % 2 == 0:
                            scalar.wait_ge(
                                s_mult, mult_count(b, min(k + 1, nact_red - 1))
                            )
                        if k < te:
                            tgt = eproj[b % 2].ap()[:, k : k + 1]
                        else:
                            tgt = dproj[b % 2].ap()[:, k - te : k - te + 1]
                        g = nprod * b + k
                        nc.scalar.activation(
                            out=scr[g % NSCR].ap(),
                            in_=scr[g % NSCR].ap(),
                            func=mybir.ActivationFunctionType.Copy,
                            accum_out=tgt,
                        ).then_inc(s_acc, 1)
                    # enc_proj row: PSUM -> SBUF, permuted to p-major order,
                    # with the mlp bias folded in.
                    if b == 0:
                        scalar.wait_ge(s_gp, 48)
                    scalar.wait_ge(s_pe, 2 * b + 1)
                    nc.scalar.add(
                        enc_row[b % 2].ap().rearrange("o (p t) -> o p t", p=P),
                        tp_row.ap().rearrange("o (t p) -> o p t", p=P),
                        add=bias_b.ap()[0:1, 0:1],
                    ).then_inc(s_acc, 1)
                    # output builds: out = ebc + dec_proj (per-partition bias).
                    scalar.wait_ge(s_pe, 2 * b + 2)
                    if b >= 2:
                        scalar.wait_ge(s_outdma[b % 2], 16 * (b // 2))
                    for t in range(td):
                        k = te + t
                        if k >= nact_red:
                            # this dproj column comes from DVE's reduce
                            scalar.wait_ge(s_mult, mult_count(b, k) + 1)
                        nc.scalar.add(
                            out_t[b % 2].ap()[:, t, :],
                            ebc.ap(),
                            add=dproj[b % 2].ap()[:, t : t + 1],
                        ).then_inc(s_acc, 1)

            @block.tensor
            def _(pe):
                for b in range(bpc):
                    if b == 0:
                        pe.wait_ge(s_gp, 80)  # ident + ones ready
                    pe.wait_ge(s_acc, A * b + te)  # eproj columns ready
                    last = None
                    for t in range(te):
                        last = nc.tensor.transpose(
                            tp_row.ap()[0:1, t * P : (t + 1) * P],
                            eproj[b % 2].ap()[:, t : t + 1],
                            ident.ap(),
                        )
                    last.then_inc(s_pe, 1)
                    pe.wait_ge(s_acc, A * b + nact_red + 1)  # enc_row ready
                    last = None
                    for h in range(nh):
                        last = nc.tensor.matmul(
                            ebc.ap()[:, h * nblk : (h + 1) * nblk],
                            ones_row.ap(),
                            enc_row[b % 2].ap()[0:1, h * nblk : (h + 1) * nblk],
                            start=True,
                            stop=True,
                        )
                    last.then_inc(s_pe, 1)

    return nc


_NC_CACHE = {}


def _get_nc():
    if "nc" not in _NC_CACHE:
        _NC_CACHE["nc"] = _build()
    return _NC_CACHE["nc"]


_IDENT = np.eye(P, dtype=np.float32)
_ONES = np.ones((1, P), dtype=np.float32)


def _shard_inputs(decoder_states, encoder_states, mlp_weight, mlp_bias):
    decoder_states = np.ascontiguousarray(np.asarray(decoder_states, dtype=np.float32))
    encoder_states = np.ascontiguousarray(np.asarray(encoder_states, dtype=np.float32))
    mlp_weight = np.asarray(mlp_weight, dtype=np.float32).reshape(1, 2 * DIM)
    mlp_bias = np.ascontiguousarray(
        np.asarray(mlp_bias, dtype=np.float32).reshape(1, 1)
    )

    w_enc = np.ascontiguousarray(mlp_weight[:, :DIM])
    w_dec = np.ascontiguousarray(mlp_weight[:, DIM:])

    in_maps = []
    for i in range(NCORES):
        lo = i * BPC
        in_maps.append(
            {
                "dec_in": decoder_states[lo : lo + BPC].reshape(BPC * DEC, DIM),
                "enc_in": encoder_states[lo : lo + BPC].reshape(BPC * ENC, DIM),
                "w_enc": w_enc,
                "w_dec": w_dec,
                "bias": mlp_bias,
                "ident_in": _IDENT,
                "ones_in": _ONES,
            }
        )
    return in_maps


def _gather(res):
    shards = [r["out"].reshape(BPC, DEC, ENC) for r in res.results]
    return np.concatenate(shards, axis=0)


def kernel(decoder_states, encoder_states, step, mlp_weight, mlp_bias, **_ignored):
    in_maps = _shard_inputs(decoder_states, encoder_states, mlp_weight, mlp_bias)
    res = run_bass_kernel_spmd(_get_nc(), in_maps, core_ids=list(range(NCORES)))
    return _gather(res)



# revision 6
# speedup vs baseline: 1.6205x; 1.6205x over previous
"""Trainium2 Bass kernel for nn_Concat_73607149519362.

Math (decomposed concat-MLP attention score):
    score[b, d, e] = dec[b, d] @ w_dec + enc[b, e] @ w_enc + bias

Sharding: data-parallel over batch, 32 batches / 8 cores = 4 per core.

v2 design — fp16 I/O (halves HBM traffic vs f32), projections on PE:
  - Host ships enc transposed (dim-major) in fp16.  PE computes
    eproj = w_enc^T @ enc_T as 8 accumulating K=128 matmuls per
    512-column half -> PSUM row [1, enc].
  - ACT copies that row to SBUF fp16 with the mlp bias folded in; PE
    broadcasts it to ebc [128, enc] PSUM via a ones-column matmul.
  - dec ships row-major fp16; DVE computes dproj columns with fused
    tensor_tensor_reduce (mult + free-axis add) against a broadcast
    w_dec row.
  - ACT builds out chunks: activation(Identity, in_=ebc,
    bias=dproj column) writing fp16 SBUF, then issues the output DMA
    on its own HW DGE queue (dec input DMA rides the same queue).
  - enc input DMAs ride the SP HW DGE queue.  gpsimd only preloads
    weights.  All DRAM views are p-major: each partition reads/writes
    one contiguous 4-16KB run per transfer.
"""

import os
from contextlib import ExitStack

os.environ.setdefault("JAX_PLATFORMS", "axon")

import numpy as np

import concourse.bass as bass
import concourse.mybir as mybir
from concourse.bass_utils import run_bass_kernel_spmd

B, DEC, ENC, DIM = 32, 512, 1024, 1024
NCORES = 8
BPC = B // NCORES  # batches per core

F16 = mybir.dt.float16
F32 = mybir.dt.float32
P = 128
TE = DIM // P  # enc contraction slots (dim-major)
TD = DEC // P  # dec 128-row chunks
EG = [(0, 1), (1, 2), (2, 4), (4, 6), (6, 8)]  # enc DMA slot groups
HALF = ENC // 2


def _build(bpc=BPC, dec=DEC, enc=ENC, dim=DIM):
    nc = bass.Bass("TRN2")
    enc_h = nc.dram_tensor("enc_in", [bpc * dim, enc], F16, kind="ExternalInput")
    dec_h = nc.dram_tensor("dec_in", [bpc * dec, dim], F16, kind="ExternalInput")
    wenc_h = nc.dram_tensor("w_enc", [P, TE], F16, kind="ExternalInput")
    wdec_h = nc.dram_tensor("w_dec", [1, dim], F16, kind="ExternalInput")
    bias_h = nc.dram_tensor("bias", [1, 1], F32, kind="ExternalInput")
    ones_h = nc.dram_tensor("ones_in", [1, P], F16, kind="ExternalInput")
    out_h = nc.dram_tensor("out", [bpc * dec, enc], F16, kind="ExternalOutput")

    # p-major DRAM views: one contiguous run per partition per transfer.
    enc_r = enc_h.ap().rearrange("(b p t) e -> b p t e", p=P, t=TE)
    dec_r = dec_h.ap().rearrange("(b p t) d -> b p t d", p=P, t=TD)
    out_r = out_h.ap().rearrange("(b p t) e -> b p t e", p=P, t=TD)

    with ExitStack() as ctx:

        def sb(name, shape, dt=F16):
            return ctx.enter_context(nc.sbuf_tensor(name, shape, dt))

        w_enc_sb = sb("w_enc_sb", [P, TE])
        w_dec_b = sb("w_dec_b", [P, dim])
        ones_sb = sb("ones_sb", [1, P])
        bias_b = sb("bias_b", [1, 1], F32)
        enc_t = [sb(f"enc_t{i}", [P, TE, enc]) for i in range(2)]
        dec_t = [sb(f"dec_t{i}", [P, TD, dim]) for i in range(2)]
        out_t = [sb(f"out_t{i}", [P, TD, enc]) for i in range(2)]
        scr = sb("scr", [P, dim])
        eproj_sb = [sb(f"eproj_sb{i}", [1, enc]) for i in range(2)]
        dproj_sb = [sb(f"dproj_sb{i}", [P, TD], F32) for i in range(2)]
        eproj_ps = ctx.enter_context(nc.psum_tensor("eproj_ps", [1, enc], F32))
        ebc = ctx.enter_context(nc.psum_tensor("ebc", [P, enc], F32))

        s_gp = ctx.enter_context(nc.semaphore(name="s_gp"))
        s_enc = [
            [ctx.enter_context(nc.semaphore(name=f"s_enc{i}_{g}")) for g in range(len(EG))]
            for i in range(2)
        ]
        s_dec = [ctx.enter_context(nc.semaphore(name=f"s_dec{i}")) for i in range(2)]
        s_pe = ctx.enter_context(nc.semaphore(name="s_pe"))
        s_acc = ctx.enter_context(nc.semaphore(name="s_acc"))
        s_mul = ctx.enter_context(nc.semaphore(name="s_mul"))
        s_ttr = ctx.enter_context(nc.semaphore(name="s_ttr"))
        s_bld = ctx.enter_context(nc.semaphore(name="s_bld"))
        s_out = [ctx.enter_context(nc.semaphore(name=f"s_out{i}")) for i in range(2)]

        with nc.Block(no_gpsimd_drain=True) as block:

            @block.gpsimd
            def _(gpsimd):
                gpsimd.dma_start(w_enc_sb.ap(), wenc_h.ap()).then_inc(s_gp, 16)
                gpsimd.dma_start(
                    w_dec_b.ap(), wdec_h.ap().to_broadcast((P, dim))
                ).then_inc(s_gp, 16)
                gpsimd.dma_start(ones_sb.ap(), ones_h.ap()).then_inc(s_gp, 16)
                gpsimd.dma_start(bias_b.ap(), bias_h.ap()).then_inc(s_gp, 16)

            @block.sync
            def _(sync):
                issued = []  # (sem, val) completion points: cap queue depth
                for b in range(bpc):
                    buf, dv = b % 2, 16 * (b // 2 + 1)
                    if b >= 2:
                        # WAR: enc_t[buf] free once PE finished b-2's matmuls
                        sync.wait_ge(s_pe, 2 * (b - 2) + 1)
                    for g, (lo, hi) in enumerate(EG):
                        if len(issued) >= 3:
                            psem, pval = issued[-3]
                            sync.wait_ge(psem, pval)
                        sync.dma_start(
                            enc_t[buf].ap()[:, lo:hi, :], enc_r[b][:, lo:hi, :]
                        ).then_inc(s_enc[buf][g], 16)
                        issued.append((s_enc[buf][g], dv))

            @block.tensor
            def _(pe):
                pe.wait_ge(s_gp, 64)
                for b in range(bpc):
                    buf, dv = b % 2, 16 * (b // 2 + 1)
                    if b >= 1:
                        # eproj_ps drained by ACT's copy of b-1
                        pe.wait_ge(s_acc, b)
                    last = None
                    for t in range(TE):
                        for g, (lo, hi) in enumerate(EG):
                            if t == lo:
                                pe.wait_ge(s_enc[buf][g], dv)
                        st, sp = (t == 0), (t == TE - 1)
                        nc.tensor.matmul(
                            eproj_ps.ap()[0:1, 0:HALF],
                            w_enc_sb.ap()[:, t : t + 1],
                            enc_t[buf].ap()[:, t, 0:HALF],
                            start=st,
                            stop=sp,
                        )
                        last = nc.tensor.matmul(
                            eproj_ps.ap()[0:1, HALF:enc],
                            w_enc_sb.ap()[:, t : t + 1],
                            enc_t[buf].ap()[:, t, HALF:enc],
                            start=st,
                            stop=sp,
                        )
                    last.then_inc(s_pe, 1)  # -> 2b+1: eproj ready, enc_t free
                    pe.wait_ge(s_acc, b + 1)  # eproj_sb[buf] written
                    if b >= 1:
                        pe.wait_ge(s_bld, 4 * b)  # ebc free: b-1 builds done
                    nc.tensor.matmul(
                        ebc.ap()[:, 0:HALF],
                        ones_sb.ap(),
                        eproj_sb[buf].ap()[0:1, 0:HALF],
                        start=True,
                        stop=True,
                    )
                    nc.tensor.matmul(
                        ebc.ap()[:, HALF:enc],
                        ones_sb.ap(),
                        eproj_sb[buf].ap()[0:1, HALF:enc],
                        start=True,
                        stop=True,
                    ).then_inc(s_pe, 1)  # -> 2b+2: ebc ready

            @block.vector
            def _(vector):
                vector.wait_ge(s_gp, 64)
                for b in range(bpc):
                    buf, dv = b % 2, 16 * (b // 2 + 1)
                    vector.wait_ge(s_dec[buf], dv)
                    if b >= 2:
                        # dproj_sb[buf] free once b-2's builds consumed it
                        vector.wait_ge(s_bld, 4 * (b - 2) + 4)
                    for c in range(TD):
                        g = TD * b + c
                        nc.vector.tensor_tensor(
                            out=scr.ap(),
                            in0=dec_t[buf].ap()[:, c, :],
                            in1=w_dec_b.ap(),
                            op=mybir.AluOpType.mult,
                        ).then_inc(s_mul, 1)
                        # self-wait: the multiply's SBUF writes must retire
                        # before the reduce reads them (DVE pipelines acks)
                        vector.wait_ge(s_mul, g + 1)
                        nc.vector.tensor_reduce(
                            out=dproj_sb[buf].ap()[:, c : c + 1],
                            in_=scr.ap(),
                            axis=mybir.AxisListType.X,
                            op=mybir.AluOpType.add,
                        ).then_inc(s_ttr, 1)

            @block.scalar
            def _(scalar):
                scalar.wait_ge(s_gp, 64)
                scalar.dma_start(dec_t[0].ap(), dec_r[0]).then_inc(s_dec[0], 16)
                for b in range(bpc):
                    buf = b % 2
                    if b + 1 < bpc:
                        nbuf = (b + 1) % 2
                        if b >= 1:
                            # WAR: dec_t[nbuf] free once b-1's TTRs done
                            scalar.wait_ge(s_ttr, 4 * b)
                        scalar.dma_start(dec_t[nbuf].ap(), dec_r[b + 1]).then_inc(
                            s_dec[nbuf], 16
                        )
                    scalar.wait_ge(s_pe, 2 * b + 1)
                    # eproj row: PSUM -> SBUF fp16 with mlp bias folded in
                    nc.scalar.add(
                        eproj_sb[buf].ap(), eproj_ps.ap(), add=bias_b.ap()[0:1, 0:1]
                    ).then_inc(s_acc, 1)
                    scalar.wait_ge(s_pe, 2 * b + 2)  # ebc ready
                    if b >= 2:
                        # out_t[buf] free once b-2's output DMAs completed
                        scalar.wait_ge(s_out[buf], 32 * (b // 2))
                    for c in range(TD):
                        scalar.wait_ge(s_ttr, 4 * b + c + 1)
                        nc.scalar.add(
                            out_t[buf].ap()[:, c, :],
                            ebc.ap(),
                            add=dproj_sb[buf].ap()[:, c : c + 1],
                        ).then_inc(s_bld, 1)
                        if c == 1:
                            # self-wait: build writes must drain before DGE reads
                            scalar.wait_ge(s_bld, 4 * b + 2)
                            scalar.dma_start(
                                out_r[b][:, 0:2, :], out_t[buf].ap()[:, 0:2, :]
                            ).then_inc(s_out[buf], 16)
                        elif c == 3:
                            scalar.wait_ge(s_bld, 4 * b + 4)
                            scalar.dma_start(
                                out_r[b][:, 2:4, :], out_t[buf].ap()[:, 2:4, :]
                            ).then_inc(s_out[buf], 16)
                # ensure all output DMAs landed before the block drains
                scalar.wait_ge(s_out[0], 32 * ((bpc + 1) // 2))
                scalar.wait_ge(s_out[1], 32 * (bpc // 2))

    return nc


_NC_CACHE = {}


def _get_nc():
    if "nc" not in _NC_CACHE:
        _NC_CACHE["nc"] = _build()
    return _NC_CACHE["nc"]


_ONES = np.ones((1, P), dtype=np.float16)


def _shard_inputs(decoder_states, encoder_states, mlp_weight, mlp_bias):
    dec16 = np.asarray(decoder_states, dtype=np.float16)
    enc16t = np.asarray(encoder_states, dtype=np.float16).transpose(0, 2, 1)
    w = np.asarray(mlp_weight, dtype=np.float16).reshape(2 * DIM)
    w_enc = np.ascontiguousarray(w[:DIM].reshape(P, TE))
    w_dec = np.ascontiguousarray(w[DIM:].reshape(1, DIM))
    bias = np.asarray(mlp_bias, dtype=np.float32).reshape(1, 1)

    in_maps = []
    for i in range(NCORES):
        lo = i * BPC
        in_maps.append(
            {
                "enc_in": np.ascontiguousarray(enc16t[lo : lo + BPC]).reshape(
                    BPC * DIM, ENC
                ),
                "dec_in": np.ascontiguousarray(dec16[lo : lo + BPC]).reshape(
                    BPC * DEC, DIM
                ),
                "w_enc": w_enc,
                "w_dec": w_dec,
                "bias": bias,
                "ones_in": _ONES,
            }
        )
    return in_maps


def _gather(res):
    shards = [r["out"].reshape(BPC, DEC, ENC) for r in res.results]
    return np.concatenate(shards, axis=0).astype(np.float32)


def kernel(decoder_states, encoder_states, step, mlp_weight, mlp_bias, **_ignored):
    in_maps = _shard_inputs(decoder_states, encoder_states, mlp_weight, mlp_bias)
    res = run_bass_kernel_spmd(_get_nc(), in_maps, core_ids=list(range(NCORES)))
    return _gather(res)
